# revision 24
# baseline (speedup 1.0000x reference)
"""Trainium2 Bass kernel: quadrant-stack 1x1-conv (dense_cnn).

Math (per batch b):
    f_all = channel-concat of the 4 spatial quadrants of x  -> [4C, h, w]
    g     = w_conv @ f_all (1x1 conv == channel mixing)     -> [4C, h, w]
    y quadrants: TL<-g[0:C], BL<-g[C:2C], TR<-g[2C:3C], BR<-g[3C:4C]

Distribution: data-parallel over batch across 8 NeuronCores (2 batches
per core); the 256x256 weight is replicated.

Per-core layout trick: an SBUF tile [128, R, 256] holding, for R
quadrant-rows, the full-width top rows on partitions 0:64 and the
full-width bottom rows on partitions 64:128 simultaneously provides
both K-chunks of the channel-stacked activation:
    cols   0:128 -> K-chunk 0 (TL channels on p0:64, BL on p64:128)
    cols 128:256 -> K-chunk 1 (TR, BR)
All DMAs move full-width rows, i.e. contiguous R-row runs per channel.

I/O precision: host casts x/w to fp16 and upcasts the fp16 result back
to fp32 (measured end-to-end max-err 4.8e-4 of output absmax vs the
fp32 reference) - this halves both DMA streams. Set BASS_IO_DTYPE=f32
for full-fp32 I/O (fp32r matmul, 1.5e-4).

Input loads ride the sync engine's HWDGE ring, output stores the scalar
engine's, so stores never head-of-line-block loads.
"""

import os
import sys

import numpy as np

# concourse (bass) normally arrives via the container's sitecustomize
# path setup; keep a fallback for bare environments
try:  # noqa: SIM105
    import concourse  # noqa: F401
except ImportError:
    for _p in ("/opt/trn_rl_repo", "/root/.axon_site/_ro/trn_rl_repo"):
        if os.path.isdir(_p) and _p not in sys.path:
            sys.path.append(_p)

B, C, H, W = 16, 64, 256, 256
N_CORES = 8
B_LOC = B // N_CORES          # 2 batches per core
HQ, WQ = H // 2, W // 2       # 128x128 quadrants
K = 4 * C                     # 256 channels after quadrant stacking

IO_DTYPE = os.environ.get("BASS_IO_DTYPE", "f16")   # f16 | bf16 | f32
ROWS_PER_TILE = int(os.environ.get("BASS_ROWS_PER_TILE", "8"))
PSUM_BUFS = int(os.environ.get("BASS_PSUM_BUFS", "2"))
ACT_COPY_ROWS = int(os.environ.get("BASS_ACT_COPY_ROWS", "0"))
WARMUP_MMS = int(os.environ.get("BASS_WARMUP_MMS", "20"))
IO_BUFS = int(os.environ.get("BASS_IO_BUFS", "8"))

_CACHE = {}


def _dts(io_dtype):
    import concourse.mybir as mybir

    return {
        "f16": (mybir.dt.float16, np.float16),
        "bf16": (mybir.dt.bfloat16, None),  # np dtype filled lazily
        "f32": (mybir.dt.float32r, np.float32),
    }[io_dtype]


def _np_dt(io_dtype):
    if io_dtype == "bf16":
        import ml_dtypes

        return ml_dtypes.bfloat16
    return {"f16": np.float16, "f32": np.float32}[io_dtype]


def _build(io_dtype: str, rows: int):
    import concourse.mybir as mybir
    import concourse.tile as tile
    from concourse import bacc

    f32 = mybir.dt.float32
    fio = _dts(io_dtype)[0]
    R = rows
    assert HQ % R == 0 and R % 4 == 0

    nc = bacc.Bacc(target_bir_lowering=False)
    x = nc.declare_dram_parameter("x", [B_LOC, C, H, W], fio, isOutput=False)
    wt = nc.declare_dram_parameter("wt", [K, K], fio, isOutput=False)
    y = nc.declare_dram_parameter("y", [B_LOC, C, H, W], fio, isOutput=True)

    # rows of quadrant-space per PSUM bank (bank = 2KB/partition = 512 f32)
    rows_per_bank = 4

    with tile.TileContext(nc) as tc:
        with (
            tc.tile_pool(name="wp", bufs=1) as wp,
            tc.tile_pool(name="inp", bufs=IO_BUFS) as inp,
            tc.tile_pool(name="outp", bufs=IO_BUFS) as outp,
            tc.tile_pool(name="psp", bufs=PSUM_BUFS, space="PSUM") as psp,
        ):
            # wt_sb[p, kc, m] = w_conv.T[kc*128+p, m] = w_conv[m, kc*128+p]
            wt_sb = wp.tile([128, 2, K], fio)
            nc.sync.dma_start(wt_sb[:, :, :], wt.rearrange("(kc p) m -> p kc m", p=128))

            if WARMUP_MMS:
                # dummy matmuls overlapping the first input loads: pulls the
                # PE HAM clock-gate to 8/8 before the real stream begins
                wu_w = wp.tile([128, 128], fio, name="wu_w")
                wu_x = wp.tile([128, 512], fio, name="wu_x")
                nc.gpsimd.memset(wu_w[:, :], 0.0)
                nc.gpsimd.memset(wu_x[:, :], 0.0)
                wu_ps = psp.tile([128, 512], f32, name="wu_ps", tag="ps0")
                for _ in range(WARMUP_MMS):
                    nc.tensor.matmul(wu_ps[:, :], wu_w[:, :], wu_x[:, :],
                                     start=True, stop=True)

            def row_schedule(b):
                # small tiles at the kernel's global start (fills the output
                # pipeline sooner) and global end (drains it sooner)
                lead, trail = [], []
                if R > rows_per_bank:
                    if b == 0:
                        lead = [rows_per_bank, rows_per_bank]
                    if b == B_LOC - 1:
                        trail = [rows_per_bank, rows_per_bank]
                body = (HQ - sum(lead) - sum(trail)) // R
                rows = lead + [R] * body + trail
                assert sum(rows) == HQ
                spans, r0 = [], 0
                for nr in rows:
                    spans.append((r0, nr))
                    r0 += nr
                return spans

            for b in range(B_LOC):
                for r0, nr in row_schedule(b):
                    tin = inp.tile([128, R, W], fio, tag="tin")
                    nc.sync.dma_start(tin[0:C, 0:nr], x[b, :, r0:r0 + nr, :])
                    nc.sync.dma_start(tin[C:2 * C, 0:nr],
                                      x[b, :, HQ + r0:HQ + r0 + nr, :])
                    tout = outp.tile([128, R, W], fio, tag="tout")
                    pss = [psp.tile([128, R, 128], f32, tag=f"ps{m}", name=f"ps{m}")
                           for m in range(2)]
                    # kc outer: stationary weight reused across all bank-
                    # matmuls; same-bank accumulate pairs are 4 apart
                    for kc in range(2):
                        for m in range(2):
                            for sub in range(nr // rows_per_bank):
                                rs = slice(sub * rows_per_bank, (sub + 1) * rows_per_bank)
                                nc.tensor.matmul(
                                    pss[m][:, rs, :],
                                    wt_sb[:, kc, m * 128:(m + 1) * 128],
                                    tin[:, rs, kc * 128:(kc + 1) * 128],
                                    start=(kc == 0),
                                    stop=(kc == 1),
                                )
                    for m in range(2):
                        nc.vector.tensor_copy(
                            tout[:, 0:nr, m * 128:(m + 1) * 128], pss[m][:, 0:nr, :]
                        )
                    # outputs on the scalar engine's HWDGE ring: keeps the
                    # sync ring free for input loads (no head-of-line block)
                    nc.scalar.dma_start(y[b, :, r0:r0 + nr, :], tout[0:C, 0:nr])
                    nc.scalar.dma_start(y[b, :, HQ + r0:HQ + r0 + nr, :],
                                        tout[C:2 * C, 0:nr])
    nc.compile()
    return nc


def _get_nc():
    key = (IO_DTYPE, ROWS_PER_TILE, PSUM_BUFS, ACT_COPY_ROWS, WARMUP_MMS, IO_BUFS)
    if key not in _CACHE:
        _CACHE[key] = _build(IO_DTYPE, ROWS_PER_TILE)
    return _CACHE[key]


def _in_maps(x: np.ndarray, w_conv: np.ndarray):
    np_dt = _np_dt(IO_DTYPE)
    x = np.ascontiguousarray(np.asarray(x, dtype=np.float32)).astype(np_dt)
    wt = np.ascontiguousarray(np.asarray(w_conv, dtype=np.float32).T).astype(np_dt)
    return [
        {"x": x[i * B_LOC:(i + 1) * B_LOC], "wt": wt} for i in range(N_CORES)
    ]


def _run(x: np.ndarray, w_conv: np.ndarray, trace: bool = False, **kw):
    from concourse.bass_utils import run_bass_kernel_spmd

    nc = _get_nc()
    res = run_bass_kernel_spmd(nc, _in_maps(x, w_conv), list(range(N_CORES)),
                               trace=trace, **kw)
    out = np.concatenate(
        [np.asarray(r["y"], dtype=np.float32) for r in res.results], axis=0
    )
    return out, res


def kernel(x: np.ndarray, w_conv: np.ndarray) -> np.ndarray:
    out, _ = _run(x, w_conv)
    return out


# revision 25
# speedup vs baseline: 1.0109x; 1.0109x over previous
"""Trainium2 Bass kernel: quadrant-stack 1x1-conv (dense_cnn).

Math (per batch b):
    f_all = channel-concat of the 4 spatial quadrants of x  -> [4C, h, w]
    g     = w_conv @ f_all (1x1 conv == channel mixing)     -> [4C, h, w]
    y quadrants: TL<-g[0:C], BL<-g[C:2C], TR<-g[2C:3C], BR<-g[3C:4C]

Distribution: data-parallel over batch across 8 NeuronCores (2 batches
per core); the 256x256 weight is replicated.

Per-core layout trick: an SBUF tile [128, R, 256] holding, for R
quadrant-rows, the full-width top rows on partitions 0:64 and the
full-width bottom rows on partitions 64:128 simultaneously provides
both K-chunks of the channel-stacked activation:
    cols   0:128 -> K-chunk 0 (TL channels on p0:64, BL on p64:128)
    cols 128:256 -> K-chunk 1 (TR, BR)
All DMAs move full-width rows, i.e. contiguous R-row runs per channel.

I/O precision: host casts x/w to fp16 and upcasts the fp16 result back
to fp32 (measured end-to-end max-err 4.8e-4 of output absmax vs the
fp32 reference) - this halves both DMA streams. Set BASS_IO_DTYPE=f32
for full-fp32 I/O (fp32r matmul, 1.5e-4).

Input loads ride the sync engine's HWDGE ring, output stores the scalar
engine's, so stores never head-of-line-block loads.
"""

import os
import sys

import numpy as np

# concourse (bass) normally arrives via the container's sitecustomize
# path setup; keep a fallback for bare environments
try:  # noqa: SIM105
    import concourse  # noqa: F401
except ImportError:
    for _p in ("/opt/trn_rl_repo", "/root/.axon_site/_ro/trn_rl_repo"):
        if os.path.isdir(_p) and _p not in sys.path:
            sys.path.append(_p)

B, C, H, W = 16, 64, 256, 256
N_CORES = 8
B_LOC = B // N_CORES          # 2 batches per core
HQ, WQ = H // 2, W // 2       # 128x128 quadrants
K = 4 * C                     # 256 channels after quadrant stacking

IO_DTYPE = os.environ.get("BASS_IO_DTYPE", "f16")   # f16 | bf16 | f32
ROWS_PER_TILE = int(os.environ.get("BASS_ROWS_PER_TILE", "8"))
PSUM_BUFS = int(os.environ.get("BASS_PSUM_BUFS", "2"))
ACT_COPY_ROWS = int(os.environ.get("BASS_ACT_COPY_ROWS", "0"))
WARMUP_MMS = int(os.environ.get("BASS_WARMUP_MMS", "20"))
IO_BUFS = int(os.environ.get("BASS_IO_BUFS", "8"))

_CACHE = {}


def _dts(io_dtype):
    import concourse.mybir as mybir

    return {
        "f16": (mybir.dt.float16, np.float16),
        "bf16": (mybir.dt.bfloat16, None),  # np dtype filled lazily
        "f32": (mybir.dt.float32r, np.float32),
    }[io_dtype]


def _np_dt(io_dtype):
    if io_dtype == "bf16":
        import ml_dtypes

        return ml_dtypes.bfloat16
    return {"f16": np.float16, "f32": np.float32}[io_dtype]


def _build(io_dtype: str, rows: int):
    import concourse.mybir as mybir
    import concourse.tile as tile
    from concourse import bacc

    f32 = mybir.dt.float32
    fio = _dts(io_dtype)[0]
    R = rows
    assert HQ % R == 0 and R % 4 == 0

    nc = bacc.Bacc(target_bir_lowering=False)
    x = nc.declare_dram_parameter("x", [B_LOC, C, H, W], fio, isOutput=False)
    wt = nc.declare_dram_parameter("wt", [K, K], fio, isOutput=False)
    y = nc.declare_dram_parameter("y", [B_LOC, C, H, W], fio, isOutput=True)

    # rows of quadrant-space per PSUM bank (bank = 2KB/partition = 512 f32)
    rows_per_bank = 4

    with tile.TileContext(nc) as tc:
        with (
            tc.tile_pool(name="wp", bufs=1) as wp,
            tc.tile_pool(name="inp", bufs=IO_BUFS) as inp,
            tc.tile_pool(name="outp", bufs=IO_BUFS) as outp,
            tc.tile_pool(name="psp", bufs=PSUM_BUFS, space="PSUM") as psp,
        ):
            # wt_sb[p, kc, m] = w_conv.T[kc*128+p, m] = w_conv[m, kc*128+p]
            wt_sb = wp.tile([128, 2, K], fio)
            nc.sync.dma_start(wt_sb[:, :, :], wt.rearrange("(kc p) m -> p kc m", p=128))

            if WARMUP_MMS:
                # dummy matmuls overlapping the first input loads: pulls the
                # PE HAM clock-gate to 8/8 before the real stream begins
                wu_w = wp.tile([128, 128], fio, name="wu_w")
                wu_x = wp.tile([128, 512], fio, name="wu_x")
                nc.gpsimd.memset(wu_w[:, :], 0.0)
                nc.gpsimd.memset(wu_x[:, :], 0.0)
                wu_ps = psp.tile([128, 512], f32, name="wu_ps", tag="ps0")
                for _ in range(WARMUP_MMS):
                    nc.tensor.matmul(wu_ps[:, :], wu_w[:, :], wu_x[:, :],
                                     start=True, stop=True)

            for b in range(B_LOC):
                for rt in range(HQ // R):
                    r0 = rt * R
                    tin = inp.tile([128, R, W], fio, tag="tin")
                    nc.sync.dma_start(tin[0:C], x[b, :, r0:r0 + R, :])
                    nc.sync.dma_start(tin[C:2 * C], x[b, :, HQ + r0:HQ + r0 + R, :])
                    tout = outp.tile([128, R, W], fio, tag="tout")
                    pss = [psp.tile([128, R, 128], f32, tag=f"ps{m}", name=f"ps{m}")
                           for m in range(2)]
                    # kc outer: stationary weight reused across all bank-
                    # matmuls; same-bank accumulate pairs are 4 apart
                    for kc in range(2):
                        for m in range(2):
                            for sub in range(R // rows_per_bank):
                                rs = slice(sub * rows_per_bank, (sub + 1) * rows_per_bank)
                                nc.tensor.matmul(
                                    pss[m][:, rs, :],
                                    wt_sb[:, kc, m * 128:(m + 1) * 128],
                                    tin[:, rs, kc * 128:(kc + 1) * 128],
                                    start=(kc == 0),
                                    stop=(kc == 1),
                                )
                    for m in range(2):
                        nc.vector.tensor_copy(
                            tout[:, :, m * 128:(m + 1) * 128], pss[m][:, :, :]
                        )
                    # outputs on the scalar engine's HWDGE ring: keeps the
                    # sync ring free for input loads (no head-of-line block)
                    nc.scalar.dma_start(y[b, :, r0:r0 + R, :], tout[0:C])
                    nc.scalar.dma_start(y[b, :, HQ + r0:HQ + r0 + R, :], tout[C:2 * C])
    nc.compile()
    return nc


def _get_nc():
    key = (IO_DTYPE, ROWS_PER_TILE, PSUM_BUFS, ACT_COPY_ROWS, WARMUP_MMS, IO_BUFS)
    if key not in _CACHE:
        _CACHE[key] = _build(IO_DTYPE, ROWS_PER_TILE)
    return _CACHE[key]


def _in_maps(x: np.ndarray, w_conv: np.ndarray):
    np_dt = _np_dt(IO_DTYPE)
    x = np.ascontiguousarray(np.asarray(x, dtype=np.float32)).astype(np_dt)
    wt = np.ascontiguousarray(np.asarray(w_conv, dtype=np.float32).T).astype(np_dt)
    return [
        {"x": x[i * B_LOC:(i + 1) * B_LOC], "wt": wt} for i in range(N_CORES)
    ]


def _run(x: np.ndarray, w_conv: np.ndarray, trace: bool = False, **kw):
    from concourse.bass_utils import run_bass_kernel_spmd

    nc = _get_nc()
    res = run_bass_kernel_spmd(nc, _in_maps(x, w_conv), list(range(N_CORES)),
                               trace=trace, **kw)
    out = np.concatenate(
        [np.asarray(r["y"], dtype=np.float32) for r in res.results], axis=0
    )
    return out, res


def kernel(x: np.ndarray, w_conv: np.ndarray) -> np.ndarray:
    out, _ = _run(x, w_conv)
    return out
